# revision 38
# baseline (speedup 1.0000x reference)
"""Block-local self-attention (BLOCK=128, 3-block windows + global token) on 8
Trainium2 NeuronCores.

Sharding: batch*heads = 32 (n,h) pairs -> 4 pairs per core, no cross-core comms.

Device kernel computes ONLY the unnormalized block-local attention in a
scores-transposed layout; everything tiny (global token slot, global query
row, mask, normalization) is folded into the host pre/post passes:

  - mask fold: exp(s + m_k) = exp(s) * exp(m_k), so the additive key mask
    becomes a per-key row scale of V' on the host (V' = [V | ones]).
  - per k-block slab j (32 of them): one matmul
      scoresT[k in block j, q in blocks qlo..qlo+2] = K_j^T.T @ Q^T-slice
    (contraction = 64, moving = 384 contiguous q columns of Q^T).
  - exp on ScalarE (FOUR slabs per instruction -- the whole scoresT ring
    -- PSUM -> SBUF bf16): the next QKs' ring-reuse WAR then targets
    exactly this instruction, so the scheduler's conservative
    latest-emitted wait coincides with the true dependency and the exp
    overhead amortizes; the serial cycle per 4 slabs is one exp + one
    4-QK block.
  - PV transposed: ctxT[d, q] per 128-query window accumulates in PSUM over
    the window's <=3 contributing slabs, with V'_j as the (65-col) stationary
    operand loaded once per slab; V' col 64 = exp(mask) accumulates the
    softmax denominator in the same matmuls.
  - K^T/Q^T are DMA-duplicated onto partitions 64..127 so even/odd slabs'
    K=64 QK matmuls run concurrently on disjoint PE row-halves (row tiling).
  - one flat 128-slab stream (no pair-boundary pipeline drain), a 4-bank
    scoresT ring + 4 one-window-per-bank ctx tiles (a PSUM accumulation
    group claims a whole 2KB bank), PV lagged 2 super-batches behind exp;
    the dense QK+PV blocks between exps keep re-igniting the HAM clock
    gate on their own.
  - DVE copies each finished window [65, 128] PSUM -> SBUF bf16 into
    16-window halves; DMA out on the Sync HWDGE ring (keeps the DGE
    triggers off the Activation queue that paces the exp chain).

Host post-pass divides by the denominator row, adds the global-token slot
exp(q.k0 + m0) x V'_0 for windows >= 2 (windows 0/1 contain token 0
locally), and computes token 0's full-softmax output row.
"""

import numpy as np
import ml_dtypes

N, H, T, D = 2, 16, 4000, 64
BLOCK = 128
TP = 4096            # padded token count (32 blocks)
W = 32               # number of 128-blocks
NCORES = 8
PAIRS = N * H        # 32
PPC = PAIRS // NCORES  # pairs per core
NGRP = W // 4          # output groups of 4 windows
SCALE = 1.0 / np.sqrt(np.float32(D))

_prog_cache = {}


def _qb0(s):
    # first q-block covered by slab s (3 contiguous q-blocks per slab)
    return min(max(s - 1, 0), W - 3)


def _build_program():
    if "nc" in _prog_cache:
        return _prog_cache["nc"]

    import concourse.bacc as bacc
    import concourse.mybir as mybir
    from concourse import tile

    dt = mybir.dt
    EXP = mybir.ActivationFunctionType.Exp

    nc = bacc.Bacc("TRN2", target_bir_lowering=False, debug=False,
                   num_devices=NCORES)
    qt_d = nc.dram_tensor("qt", [PPC, D, TP], dt.bfloat16,
                          kind="ExternalInput").ap()
    kt_d = nc.dram_tensor("kt", [PPC, D, TP], dt.bfloat16,
                          kind="ExternalInput").ap()
    vp_d = nc.dram_tensor("vp", [PPC, 128, W * 65], dt.bfloat16,
                          kind="ExternalInput").ap()
    out_d = nc.dram_tensor("out", [PPC, 2, 65, W * BLOCK // 2], dt.bfloat16,
                           kind="ExternalOutput").ap()

    with tile.TileContext(nc) as tc:
        with (
            tc.tile_pool(name="qt", bufs=2) as qt_pool,
            tc.tile_pool(name="kt", bufs=2) as kt_pool,
            tc.tile_pool(name="vp", bufs=2) as vp_pool,
            tc.tile_pool(name="ex", bufs=8) as ex_pool,
            tc.tile_pool(name="small", bufs=2) as small_pool,
            tc.tile_pool(name="outp", bufs=3) as out_pool,
            tc.tile_pool(name="sc", bufs=1, space="PSUM") as sc_pool,
            tc.tile_pool(name="ctx", bufs=4, space="PSUM") as ctx_pool,
        ):
            def load_pair(p):
                # K^T and Q^T are duplicated onto partitions 64..127 so that
                # even/odd slabs' QK matmuls run CONCURRENTLY on disjoint
                # row-halves of the PE array (K=64 row tiling).
                kt_t = kt_pool.tile([128, TP], dt.bfloat16, tag="kt",
                                    name=f"kt_{p}")
                qt_t = qt_pool.tile([128, TP], dt.bfloat16, tag="qt",
                                    name=f"qt_{p}")
                vp_t = vp_pool.tile([128, W * 65], dt.bfloat16, tag="vp",
                                    name=f"vp_{p}")
                nc.sync.dma_start(kt_t[0:D, :], kt_d[p])
                nc.sync.dma_start(kt_t[D:2 * D, :], kt_d[p])
                nc.sync.dma_start(qt_t[0:D, :], qt_d[p])
                nc.sync.dma_start(qt_t[D:2 * D, :], qt_d[p])
                nc.sync.dma_start(vp_t[:], vp_d[p])
                return qt_t, kt_t, vp_t

            # PE warm-up: dense N=512 matmuls on memset data un-throttle the
            # HAM clock gate (needs ~3.4us of sustained PE busy) while the
            # first pair's inputs stream in.  Output goes to the last slab
            # slot of the scoresT ring so no extra PSUM bank is needed.
            warm_sb = small_pool.tile([128, 640], dt.bfloat16, tag="warm")
            nc.vector.memset(warm_sb[:], 0.25)
            # single scoresT ring shared by every pair: 4 slab slots of 512
            # f32 (4 PSUM banks); slab g lives at slot g%4.  One global tile
            # so pair boundaries carry no pool-rotation dependency.
            sc_t = sc_pool.tile([128, 2048], dt.float32, tag="sc",
                                name="sc_ring")

            def emit_warm(n):
                # PE filler in a rotating ctx bank: emitted AFTER the first
                # QK batches so the ACT pipeline buffers results while the
                # filler keeps the PE dense (HAM clock gate opens and the
                # steady state starts gapless).
                bt = ctx_pool.tile([65, 512], dt.float32, tag="ctx",
                                   name="warm_ps")
                for r in range(n):
                    nc.tensor.matmul(bt[:], warm_sb[:, 0:65],
                                     warm_sb[:, 128:640], start=True,
                                     stop=True)

            # One flat stream of PPC*W slabs: the pipeline never drains at
            # pair boundaries, so once the HAM clock gate opens the PE stays
            # dense enough to keep it open.
            NSLAB = PPC * W          # 128
            NB = NSLAB // 2          # ACT batches of 2 slabs

            ptiles = {0: load_pair(0)}
            ex_tiles = {}
            ctx_tiles = {}
            out_tiles = {}

            def emit_qk(g):
                p, s = divmod(g, W)
                qt_t, kt_t, _ = ptiles[p]
                lo = _qb0(s) * BLOCK
                rb = (g % 2) * D     # row-half of the PE array
                nc.tensor.matmul(
                    sc_t[:, (g % 4) * 512:(g % 4) * 512 + 3 * BLOCK],
                    kt_t[rb:rb + D, s * BLOCK:(s + 1) * BLOCK],
                    qt_t[rb:rb + D, lo:lo + 3 * BLOCK],
                    start=True, stop=True)

            def emit_exp(B):
                # one exp per FOUR slabs (4B..4B+3 live at slots 0..3 in
                # order): reads the whole ring, so the next QKs' ring WAR
                # targets exactly this instruction -- the scheduler's
                # conservative (latest-emitted) wait coincides with the true
                # dependency and the ACT overhead amortizes 2x
                ex = ex_pool.tile([128, 4 * 3 * BLOCK], dt.bfloat16,
                                  tag="ex", name=f"ex_{B}")
                nc.scalar.activation(
                    ex[:].rearrange("p (b x) -> p b x", x=3 * BLOCK),
                    sc_t[:].rearrange("p (b x) -> p b x", x=512)[:, :, 0:3 * BLOCK],
                    EXP)
                ex_tiles[B] = ex

            def emit_pv(g):
                p, s = divmod(g, W)
                _, _, vp_t = ptiles[p]
                ex = ex_tiles[g // 4]
                exbase = (g % 4) * 3 * BLOCK
                qb0 = _qb0(s)
                for w in (s + 1, s, s - 1):
                    if not (0 <= w < W):
                        continue
                    if (p, w) not in ctx_tiles:
                        # one window per tile: a PSUM accumulation group
                        # claims a whole 2KB bank (zero region), so windows
                        # cannot share a bank while accumulating
                        ctx_tiles[p, w] = ctx_pool.tile(
                            [65, 512], dt.float32, tag="ctx",
                            name=f"ctx_{p}_{w}")
                    g2 = w - qb0
                    nc.tensor.matmul(
                        ctx_tiles[p, w][:, 0:BLOCK],
                        vp_t[:, s * 65:(s + 1) * 65],
                        ex[:, exbase + g2 * BLOCK:exbase + (g2 + 1) * BLOCK],
                        start=(s == max(w - 1, 0)),
                        stop=(s == min(w + 1, W - 1)))
                if g % 4 == 3:
                    ex_tiles.pop(g // 4)
                done = [s - 1] if s < W - 1 else [W - 2, W - 1]
                for w in done:
                    if w < 0:
                        continue
                    hi, wi = w // 16, w % 16
                    if wi == 0:
                        out_tiles[p, hi] = out_pool.tile(
                            [65, 16 * BLOCK], dt.bfloat16, tag="out",
                            name=f"out_{p}_{hi}")
                    ob = out_tiles[p, hi]
                    ct = ctx_tiles.pop((p, w))
                    nc.vector.tensor_copy(ob[:, wi * BLOCK:(wi + 1) * BLOCK],
                                          ct[:, 0:BLOCK])
                    if wi == 15:
                        nc.sync.dma_start(out_d[p, hi], ob[:])
                        out_tiles.pop((p, hi))

            # prologue: first super-batch of QKs + its exp; the warmup (26
            # dense matmuls, emitted before the QKs) covers the initial DMA
            NSB = NSLAB // 4         # super-batches of 4 slabs
            emit_qk(0); emit_qk(1); emit_qk(2); emit_qk(3)
            emit_exp(0)
            for B in range(1, NSB + 2):
                if B < NSB:
                    emit_qk(4 * B); emit_qk(4 * B + 1)
                    emit_qk(4 * B + 2); emit_qk(4 * B + 3)
                    emit_exp(B)
                    # prefetch the next pair's inputs mid-pair
                    p, m = divmod(B, W // 4)
                    if m == 3 and p + 1 < PPC:
                        ptiles[p + 1] = load_pair(p + 1)
                if B - 2 >= 0:
                    for k in range(4):
                        emit_pv(4 * (B - 2) + k)

    nc.compile()
    _prog_cache["nc"] = nc
    return nc


def _prep_core_inputs(q, k, v, mask):
    """q,k,v: (PAIRS, T, D) f32; mask: (N, T) f32.  Returns list of per-core
    input dicts (bf16 device layouts)."""
    bf16 = ml_dtypes.bfloat16
    in_maps = []
    for c in range(NCORES):
        qt = np.zeros((PPC, D, TP), np.float32)
        kt = np.zeros((PPC, D, TP), np.float32)
        vp = np.zeros((PPC, 128, W * 65), np.float32)
        for pp in range(PPC):
            pair = c * PPC + pp
            n = pair // H
            qt[pp, :, :T] = q[pair].T * SCALE
            kt[pp, :, :T] = k[pair].T
            # V' = [V | ones], per-key row scaled by exp(mask) (mask fold);
            # pad rows stay 0 so pad keys contribute nothing.
            Vp = np.zeros((TP, 65), np.float32)
            Vp[:T, :D] = v[pair]
            Vp[:T, D] = 1.0
            Vp[:T] *= np.exp(mask[n])[:, None]
            vp[pp] = Vp.reshape(W, 128, 65).transpose(1, 0, 2).reshape(128, W * 65)
        in_maps.append({
            "qt": qt.astype(bf16),
            "kt": kt.astype(bf16),
            "vp": vp.astype(bf16),
        })
    return in_maps


def _unshard(results, q, k, v, mask):
    full = np.empty((PAIRS, 65, TP), np.float32)
    for c in range(NCORES):
        o = np.asarray(results[c]["out"], dtype=np.float32)  # (PPC,2,65,2048)
        o = o.reshape(PPC, 2, 65, 16, BLOCK).transpose(0, 2, 1, 3, 4)
        full[c * PPC:(c + 1) * PPC] = o.reshape(PPC, 65, TP)
    num = full[:, :D, :T]                            # (PAIRS, D, T)
    den = full[:, D, :T]                             # (PAIRS, T)

    maskp = np.repeat(mask, H, axis=0)               # (PAIRS, T)
    k0 = k[:, 0, :]                                  # (PAIRS, D)
    v0 = v[:, 0, :]                                  # (PAIRS, D)
    e0 = np.exp(np.einsum('ptd,pd->pt', q, k0) * SCALE + maskp[:, 0:1])
    # global-token slot for windows >= 2 (tokens 256+); windows 0/1 already
    # contain token 0 in their local 3-block span.
    num[:, :, 2 * BLOCK:] += v0[:, :, None] * e0[:, None, 2 * BLOCK:]
    den[:, 2 * BLOCK:] += e0[:, 2 * BLOCK:]
    out = (num / den[:, None, :]).transpose(0, 2, 1)  # (PAIRS, T, D)

    # token 0: full softmax over all keys
    gs = np.einsum('pd,ptd->pt', q[:, 0], k) * SCALE + maskp
    gs -= gs.max(axis=1, keepdims=True)
    ge = np.exp(gs)
    out[:, 0, :] = np.einsum('pt,ptd->pd', ge, v) / ge.sum(1, keepdims=True)
    return out.reshape(N, H, T, D)


def _run(inputs, trace=False, tmpdir=None):
    from concourse.bass_utils import run_bass_kernel_spmd

    q = np.asarray(inputs["query_layer"], np.float32).reshape(PAIRS, T, D)
    k = np.asarray(inputs["key_layer"], np.float32).reshape(PAIRS, T, D)
    v = np.asarray(inputs["value_layer"], np.float32).reshape(PAIRS, T, D)
    mask = np.asarray(inputs["attention_mask"], np.float32).reshape(N, T)

    nc = _build_program()
    in_maps = _prep_core_inputs(q, k, v, mask)
    res = run_bass_kernel_spmd(nc, in_maps, list(range(NCORES)),
                               trace=trace, tmpdir=tmpdir)
    return _unshard(res.results, q, k, v, mask), res


def kernel(query_layer, key_layer, value_layer, attention_mask):
    out, _ = _run({
        "query_layer": query_layer,
        "key_layer": key_layer,
        "value_layer": value_layer,
        "attention_mask": attention_mask,
    })
    return out


# revision 39
# speedup vs baseline: 1.1770x; 1.1770x over previous
"""Block-local self-attention (BLOCK=128, 3-block windows + global token) on 8
Trainium2 NeuronCores.

Sharding: batch*heads = 32 (n,h) pairs -> 4 pairs per core, no cross-core comms.

Device kernel computes ONLY the unnormalized block-local attention in a
scores-transposed layout; everything tiny (global token slot, global query
row, mask, normalization) is folded into the host pre/post passes:

  - mask fold: exp(s + m_k) = exp(s) * exp(m_k), so the additive key mask
    becomes a per-key row scale of V' on the host (V' = [V | ones]).
  - per k-block slab j (32 of them): one matmul
      scoresT[k in block j, q in blocks qlo..qlo+2] = K_j^T.T @ Q^T-slice
    (contraction = 64, moving = 384 contiguous q columns of Q^T).
  - exp on ScalarE (FOUR slabs per instruction -- the whole scoresT ring
    -- PSUM -> SBUF bf16): the next QKs' ring-reuse WAR then targets
    exactly this instruction, so the scheduler's conservative
    latest-emitted wait coincides with the true dependency and the exp
    overhead amortizes; the serial cycle per 4 slabs is one exp + one
    4-QK block.
  - PV transposed: ctxT[d, q] per 128-query window accumulates in PSUM over
    the window's <=3 contributing slabs, with V'_j as the (65-col) stationary
    operand loaded once per slab; V' col 64 = exp(mask) accumulates the
    softmax denominator in the same matmuls.
  - K^T/Q^T are DMA-duplicated onto partitions 64..127 so even/odd slabs'
    K=64 QK matmuls run concurrently on disjoint PE row-halves (row tiling).
  - one flat 128-slab stream (no pair-boundary pipeline drain), a 4-bank
    scoresT ring + 4 one-window-per-bank ctx tiles (a PSUM accumulation
    group claims a whole 2KB bank), PV lagged 2 super-batches behind exp;
    the dense QK+PV blocks between exps keep re-igniting the HAM clock
    gate on their own.
  - DVE copies each finished window [65, 128] PSUM -> SBUF bf16 into
    16-window halves; DMA out on the Sync HWDGE ring (keeps the DGE
    triggers off the Activation queue that paces the exp chain).

Host post-pass divides by the denominator row, adds the global-token slot
exp(q.k0 + m0) x V'_0 for windows >= 2 (windows 0/1 contain token 0
locally), and computes token 0's full-softmax output row.
"""

import numpy as np
import ml_dtypes

N, H, T, D = 2, 16, 4000, 64
BLOCK = 128
TP = 4096            # padded token count (32 blocks)
W = 32               # number of 128-blocks
NCORES = 8
PAIRS = N * H        # 32
PPC = PAIRS // NCORES  # pairs per core
NGRP = W // 4          # output groups of 4 windows
SCALE = 1.0 / np.sqrt(np.float32(D))

_prog_cache = {}


def _qb0(s):
    # first q-block covered by slab s (3 contiguous q-blocks per slab)
    return min(max(s - 1, 0), W - 3)


def _build_program():
    if "nc" in _prog_cache:
        return _prog_cache["nc"]

    import concourse.bacc as bacc
    import concourse.mybir as mybir
    from concourse import tile

    dt = mybir.dt
    EXP = mybir.ActivationFunctionType.Exp

    nc = bacc.Bacc("TRN2", target_bir_lowering=False, debug=False,
                   num_devices=NCORES)
    qt_d = nc.dram_tensor("qt", [PPC, D, TP], dt.bfloat16,
                          kind="ExternalInput").ap()
    kt_d = nc.dram_tensor("kt", [PPC, D, TP], dt.bfloat16,
                          kind="ExternalInput").ap()
    vp_d = nc.dram_tensor("vp", [PPC, 128, W * 65], dt.bfloat16,
                          kind="ExternalInput").ap()
    out_d = nc.dram_tensor("out", [PPC, 2, 65, W * BLOCK // 2], dt.bfloat16,
                           kind="ExternalOutput").ap()

    with tile.TileContext(nc) as tc:
        with (
            tc.tile_pool(name="qt", bufs=2) as qt_pool,
            tc.tile_pool(name="kt", bufs=2) as kt_pool,
            tc.tile_pool(name="vp", bufs=2) as vp_pool,
            tc.tile_pool(name="ex", bufs=8) as ex_pool,
            tc.tile_pool(name="small", bufs=2) as small_pool,
            tc.tile_pool(name="outp", bufs=3) as out_pool,
            tc.tile_pool(name="sc", bufs=1, space="PSUM") as sc_pool,
            tc.tile_pool(name="ctx", bufs=4, space="PSUM") as ctx_pool,
        ):
            def load_pair(p):
                # K^T and Q^T are duplicated onto partitions 64..127 so that
                # even/odd slabs' QK matmuls run CONCURRENTLY on disjoint
                # row-halves of the PE array (K=64 row tiling).
                kt_t = kt_pool.tile([128, TP], dt.bfloat16, tag="kt",
                                    name=f"kt_{p}")
                qt_t = qt_pool.tile([128, TP], dt.bfloat16, tag="qt",
                                    name=f"qt_{p}")
                vp_t = vp_pool.tile([128, W * 65], dt.bfloat16, tag="vp",
                                    name=f"vp_{p}")
                nc.sync.dma_start(kt_t[0:D, :], kt_d[p])
                nc.sync.dma_start(kt_t[D:2 * D, :], kt_d[p])
                nc.sync.dma_start(qt_t[0:D, :], qt_d[p])
                nc.sync.dma_start(qt_t[D:2 * D, :], qt_d[p])
                nc.sync.dma_start(vp_t[:], vp_d[p])
                return qt_t, kt_t, vp_t

            # PE warm-up: dense N=512 matmuls on memset data un-throttle the
            # HAM clock gate (needs ~3.4us of sustained PE busy) while the
            # first pair's inputs stream in.  Output goes to the last slab
            # slot of the scoresT ring so no extra PSUM bank is needed.
            warm_sb = small_pool.tile([128, 640], dt.bfloat16, tag="warm")
            nc.vector.memset(warm_sb[:], 0.25)
            # single scoresT ring shared by every pair: 4 slab slots of 512
            # f32 (4 PSUM banks); slab g lives at slot g%4.  One global tile
            # so pair boundaries carry no pool-rotation dependency.
            sc_t = sc_pool.tile([128, 2048], dt.float32, tag="sc",
                                name="sc_ring")

            def emit_warm(n):
                # PE filler in a rotating ctx bank: emitted AFTER the first
                # QK batches so the ACT pipeline buffers results while the
                # filler keeps the PE dense (HAM clock gate opens and the
                # steady state starts gapless).
                bt = ctx_pool.tile([65, 512], dt.float32, tag="ctx",
                                   name="warm_ps")
                for r in range(n):
                    nc.tensor.matmul(bt[:], warm_sb[:, 0:65],
                                     warm_sb[:, 128:640], start=True,
                                     stop=True)

            # One flat stream of PPC*W slabs: the pipeline never drains at
            # pair boundaries, so once the HAM clock gate opens the PE stays
            # dense enough to keep it open.
            NSLAB = PPC * W          # 128
            NB = NSLAB // 2          # ACT batches of 2 slabs

            ptiles = {0: load_pair(0)}
            ex_tiles = {}
            ctx_tiles = {}
            out_tiles = {}

            def emit_qk(g):
                p, s = divmod(g, W)
                qt_t, kt_t, _ = ptiles[p]
                lo = _qb0(s) * BLOCK
                rb = (g % 2) * D     # row-half of the PE array
                nc.tensor.matmul(
                    sc_t[:, (g % 4) * 512:(g % 4) * 512 + 3 * BLOCK],
                    kt_t[rb:rb + D, s * BLOCK:(s + 1) * BLOCK],
                    qt_t[rb:rb + D, lo:lo + 3 * BLOCK],
                    start=True, stop=True)

            def emit_exp(B):
                # one exp per FOUR slabs (4B..4B+3 live at slots 0..3 in
                # order): reads the whole ring, so the next QKs' ring WAR
                # targets exactly this instruction -- the scheduler's
                # conservative (latest-emitted) wait coincides with the true
                # dependency and the ACT overhead amortizes 2x
                ex = ex_pool.tile([128, 4 * 3 * BLOCK], dt.bfloat16,
                                  tag="ex", name=f"ex_{B}")
                nc.scalar.activation(
                    ex[:].rearrange("p (b x) -> p b x", x=3 * BLOCK),
                    sc_t[:].rearrange("p (b x) -> p b x", x=512)[:, :, 0:3 * BLOCK],
                    EXP)
                ex_tiles[B] = ex

            def emit_pv(g):
                p, s = divmod(g, W)
                _, _, vp_t = ptiles[p]
                ex = ex_tiles[g // 4]
                exbase = (g % 4) * 3 * BLOCK
                qb0 = _qb0(s)
                for w in (s + 1, s, s - 1):
                    if not (0 <= w < W):
                        continue
                    if (p, w) not in ctx_tiles:
                        # one window per tile: a PSUM accumulation group
                        # claims a whole 2KB bank (zero region), so windows
                        # cannot share a bank while accumulating
                        ctx_tiles[p, w] = ctx_pool.tile(
                            [65, 512], dt.float32, tag="ctx",
                            name=f"ctx_{p}_{w}")
                    g2 = w - qb0
                    nc.tensor.matmul(
                        ctx_tiles[p, w][:, 0:BLOCK],
                        vp_t[:, s * 65:(s + 1) * 65],
                        ex[:, exbase + g2 * BLOCK:exbase + (g2 + 1) * BLOCK],
                        start=(s == max(w - 1, 0)),
                        stop=(s == min(w + 1, W - 1)))
                if g % 4 == 3:
                    ex_tiles.pop(g // 4)
                done = [s - 1] if s < W - 1 else [W - 2, W - 1]
                for w in done:
                    if w < 0:
                        continue
                    hi, wi = w // 16, w % 16
                    if wi == 0:
                        out_tiles[p, hi] = out_pool.tile(
                            [65, 16 * BLOCK], dt.bfloat16, tag="out",
                            name=f"out_{p}_{hi}")
                    ob = out_tiles[p, hi]
                    ct = ctx_tiles.pop((p, w))
                    nc.vector.tensor_copy(ob[:, wi * BLOCK:(wi + 1) * BLOCK],
                                          ct[:, 0:BLOCK])
                    if wi == 15:
                        nc.sync.dma_start(out_d[p, hi], ob[:])
                        out_tiles.pop((p, hi))

            # prologue: first super-batch of QKs + its exp; the warmup (26
            # dense matmuls, emitted before the QKs) covers the initial DMA
            NSB = NSLAB // 4         # super-batches of 4 slabs
            emit_qk(0); emit_qk(1); emit_qk(2); emit_qk(3)
            emit_exp(0)
            for B in range(1, NSB + 2):
                if B in (2, 3, 4):
                    # small HAM igniter: opens the clock gate early; with
                    # ctx bufs=4 the rotation always lands on a completed
                    # window bank (bufs=3 would corrupt -- see memory)
                    emit_warm(2)
                if B < NSB:
                    emit_qk(4 * B); emit_qk(4 * B + 1)
                    emit_qk(4 * B + 2); emit_qk(4 * B + 3)
                    emit_exp(B)
                    # prefetch the next pair's inputs mid-pair
                    p, m = divmod(B, W // 4)
                    if m == 3 and p + 1 < PPC:
                        ptiles[p + 1] = load_pair(p + 1)
                if B - 2 >= 0:
                    for k in range(4):
                        emit_pv(4 * (B - 2) + k)

    nc.compile()
    _prog_cache["nc"] = nc
    return nc


def _prep_core_inputs(q, k, v, mask):
    """q,k,v: (PAIRS, T, D) f32; mask: (N, T) f32.  Returns list of per-core
    input dicts (bf16 device layouts)."""
    bf16 = ml_dtypes.bfloat16
    in_maps = []
    for c in range(NCORES):
        qt = np.zeros((PPC, D, TP), np.float32)
        kt = np.zeros((PPC, D, TP), np.float32)
        vp = np.zeros((PPC, 128, W * 65), np.float32)
        for pp in range(PPC):
            pair = c * PPC + pp
            n = pair // H
            qt[pp, :, :T] = q[pair].T * SCALE
            kt[pp, :, :T] = k[pair].T
            # V' = [V | ones], per-key row scaled by exp(mask) (mask fold);
            # pad rows stay 0 so pad keys contribute nothing.
            Vp = np.zeros((TP, 65), np.float32)
            Vp[:T, :D] = v[pair]
            Vp[:T, D] = 1.0
            Vp[:T] *= np.exp(mask[n])[:, None]
            vp[pp] = Vp.reshape(W, 128, 65).transpose(1, 0, 2).reshape(128, W * 65)
        in_maps.append({
            "qt": qt.astype(bf16),
            "kt": kt.astype(bf16),
            "vp": vp.astype(bf16),
        })
    return in_maps


def _unshard(results, q, k, v, mask):
    full = np.empty((PAIRS, 65, TP), np.float32)
    for c in range(NCORES):
        o = np.asarray(results[c]["out"], dtype=np.float32)  # (PPC,2,65,2048)
        o = o.reshape(PPC, 2, 65, 16, BLOCK).transpose(0, 2, 1, 3, 4)
        full[c * PPC:(c + 1) * PPC] = o.reshape(PPC, 65, TP)
    num = full[:, :D, :T]                            # (PAIRS, D, T)
    den = full[:, D, :T]                             # (PAIRS, T)

    maskp = np.repeat(mask, H, axis=0)               # (PAIRS, T)
    k0 = k[:, 0, :]                                  # (PAIRS, D)
    v0 = v[:, 0, :]                                  # (PAIRS, D)
    e0 = np.exp(np.einsum('ptd,pd->pt', q, k0) * SCALE + maskp[:, 0:1])
    # global-token slot for windows >= 2 (tokens 256+); windows 0/1 already
    # contain token 0 in their local 3-block span.
    num[:, :, 2 * BLOCK:] += v0[:, :, None] * e0[:, None, 2 * BLOCK:]
    den[:, 2 * BLOCK:] += e0[:, 2 * BLOCK:]
    out = (num / den[:, None, :]).transpose(0, 2, 1)  # (PAIRS, T, D)

    # token 0: full softmax over all keys
    gs = np.einsum('pd,ptd->pt', q[:, 0], k) * SCALE + maskp
    gs -= gs.max(axis=1, keepdims=True)
    ge = np.exp(gs)
    out[:, 0, :] = np.einsum('pt,ptd->pd', ge, v) / ge.sum(1, keepdims=True)
    return out.reshape(N, H, T, D)


def _run(inputs, trace=False, tmpdir=None):
    from concourse.bass_utils import run_bass_kernel_spmd

    q = np.asarray(inputs["query_layer"], np.float32).reshape(PAIRS, T, D)
    k = np.asarray(inputs["key_layer"], np.float32).reshape(PAIRS, T, D)
    v = np.asarray(inputs["value_layer"], np.float32).reshape(PAIRS, T, D)
    mask = np.asarray(inputs["attention_mask"], np.float32).reshape(N, T)

    nc = _build_program()
    in_maps = _prep_core_inputs(q, k, v, mask)
    res = run_bass_kernel_spmd(nc, in_maps, list(range(NCORES)),
                               trace=trace, tmpdir=tmpdir)
    return _unshard(res.results, q, k, v, mask), res


def kernel(query_layer, key_layer, value_layer, attention_mask):
    out, _ = _run({
        "query_layer": query_layer,
        "key_layer": key_layer,
        "value_layer": value_layer,
        "attention_mask": attention_mask,
    })
    return out
